# revision 1
# baseline (speedup 1.0000x reference)
"""Trainium2 Bass kernel for GQA causal attention (nn_Attention_83623013253180).

Shapes: B=2, L=2048, D=1024, H=16 heads, G=2 kv-groups, HPG=8, DQK=DV=128.

Sharding (8 cores): core c -> (b = c//4, g = (c%4)//2, hh = c%2), each core
handles one batch, one kv group, and 4 of that group's 8 query heads.
Wq/Wk/Wv are column-sharded, Wo row-sharded; the out-proj all-reduce (sum of
4 partials per batch) is done on host after gather, along with + bo.

Per-core device kernel (matmul operands fp16, PSUM fp32), pipelined over
512-token q chunks:
  - projections: kT/qT [dqk,tok] via W-stationary matmuls, v [tok,dv] via
    x-stationary matmuls; q/k/v biases are zero by construction in the
    reference, so PSUM->SBUF moves are plain copies.
  - attention per chunk runs in two head-pair passes so PSUM fits:
      ps_s    (2 bufs x 2 banks): S^T tiles [kv=128, 2 heads, q<=512]
      ps_ctx  (1 buf  x 2 banks): ctxT accumulators [dv=128, 2 heads, 512]
      ps_proj (1 buf  x 1 bank):  projection PSUM
      ps_od   (1 buf  x 1 bank):  denominators / out-proj PSUM
    Per kv tile: 2 score matmuls (k-tile stationary) -> one batched exp
    (ScalarE, [128, 2, w]) -> multiplicative 0/1 causal mask applied in
    place on the diagonal 128-block of e (DVE, off the scores->exp path) ->
    2 attnV matmuls with v-tile stationary accumulating ctxT[dv, q] in PSUM
    (no PE transposes), plus one DVE add per tile building acc_e for the
    softmax denominator (first tile written by exp directly).
    A 16-matmul warmup on scratch SBUF keeps the PE HAM clock at 8/8
    through the initial DMA wait; next-chunk xk DMAs are issued between
    attention passes; the last chunk ping-pongs out-proj/denominator PSUM
    tiles across the two spare banks to avoid a serialized tail.
  - denominator: ones[128,128]-stationary matmul over acc_e gives the
    partition-sum replicated across all 128 partitions; reciprocal via the
    fast approx custom-DVE op; DVE multiply normalizes ctxT (fp16).
  - out projection: ctxT tiles stationary, wo streaming, 4-head PSUM
    accumulation, fp16 DMA partials; host sums partials + bo in fp32.
"""

import numpy as np

import concourse.bass as bass
import concourse.mybir as mybir
import concourse.tile as tile
from concourse import bacc
from concourse.bass_utils import run_bass_kernel_spmd

F16 = mybir.dt.float16
F32 = mybir.dt.float32

B, L, D = 2, 2048, 1024
H, G, HPG = 16, 2, 8
DQK = DV = 128
NHEAD = 4          # heads per core
NDT = D // 128     # 8 contraction tiles over input dim
NKV = L // 128     # 16 kv tiles
QC = 512           # q chunk width
NQC = L // QC      # 4 q chunks
NCORES = 8


def _build(scale_val: float) -> bass.Bass:
    nc = bacc.Bacc("TRN2", target_bir_lowering=False, debug=False, num_devices=NCORES)

    xq = nc.dram_tensor("xqT", [NQC, 128, NDT, QC], F16, kind="ExternalInput")
    xk = nc.dram_tensor("xkT", [NQC, 128, NDT, QC], F16, kind="ExternalInput")
    xv = nc.dram_tensor("xvT", [NQC, 128, NDT, QC], F16, kind="ExternalInput")
    wq = nc.dram_tensor("wq", [128, NDT, NHEAD * DQK], F16, kind="ExternalInput")
    wk = nc.dram_tensor("wk", [128, NDT, DQK], F16, kind="ExternalInput")
    wv = nc.dram_tensor("wv", [128, NDT, DV], F16, kind="ExternalInput")
    wo = nc.dram_tensor("wo", [128, NHEAD, D], F16, kind="ExternalInput")
    mb = nc.dram_tensor("mb", [128, 2, 128], F16, kind="ExternalInput")
    one = nc.dram_tensor("one", [128, 128], F16, kind="ExternalInput")
    out = nc.dram_tensor("out", [L, D], F16, kind="ExternalOutput")

    with tile.TileContext(nc) as tc:
        with (
            tc.tile_pool(name="const", bufs=1) as cpool,
            tc.tile_pool(name="xbuf", bufs=1) as xpool,
            tc.tile_pool(name="qkv", bufs=1) as qkvpool,
            tc.tile_pool(name="ebuf", bufs=4) as epool,
            tc.tile_pool(name="accbuf", bufs=3) as accpool,
            tc.tile_pool(name="rbbuf", bufs=4) as rbpool,
            tc.tile_pool(name="ctxt", bufs=2) as ctpool,
            tc.tile_pool(name="outb", bufs=3) as opool,
            tc.tile_pool(name="ps_s", bufs=2, space="PSUM") as ps_s,
            tc.tile_pool(name="ps_ctx", bufs=1, space="PSUM") as ps_ctx,
            tc.tile_pool(name="ps_proj", bufs=1, space="PSUM") as ps_proj,
            tc.tile_pool(name="ps_od", bufs=1, space="PSUM") as ps_od,
        ):
            wk_sb = cpool.tile([128, NDT, DQK], F16, tag="wk")
            mb_sb = cpool.tile([128, 2, 128], F16, tag="mb")
            one_sb = cpool.tile([128, 128], F16, tag="one")
            wq_sb = cpool.tile([128, NDT, NHEAD * DQK], F16, tag="wq")
            wv_sb = cpool.tile([128, NDT, DV], F16, tag="wv")
            wo_sb = cpool.tile([128, NHEAD, D], F16, tag="wo")

            q_sb = qkvpool.tile([128, NHEAD, L], F16, tag="q")    # qT per head
            k_sb = qkvpool.tile([128, L], F16, tag="k")           # kT
            v_sb = qkvpool.tile([128, NKV, DV], F16, tag="v")     # v [tok, dv]

            xq_sb = xpool.tile([128, NQC, NDT, QC], F16, tag="xq")
            xk_sb = xpool.tile([128, NQC, NDT, QC], F16, tag="xk")
            xv_sb = xpool.tile([128, NQC, NDT, QC], F16, tag="xv")

            # ---- HAM warmup: dummy matmuls on (uninitialized) SBUF while
            # the first DMAs stream in, so the PE clock is at 8/8 when real
            # work starts. Results go to a PSUM bank that is never read.
            for wu in range(16):
                wu_ps = ps_od.tile([128, QC], F32, tag="od")
                nc.tensor.matmul(
                    wu_ps, xv_sb[:, 3, 0, 0:128], xv_sb[:, 3, wu % NDT, :],
                    start=True, stop=True,
                )

            for ch in range(NQC):
                sl = slice(ch * QC, (ch + 1) * QC)

                # ---- load + project this chunk (k, then v, then q) ----
                if ch == 0:
                    nc.sync.dma_start(wk_sb[:], wk[:])
                    nc.sync.dma_start(xk_sb[:, ch], xk[ch])
                pk = ps_proj.tile([128, QC], F32, tag="proj")
                for dt_i in range(NDT):
                    nc.tensor.matmul(
                        pk, wk_sb[:, dt_i, :], xk_sb[:, ch, dt_i, :],
                        start=(dt_i == 0), stop=(dt_i == NDT - 1),
                    )
                nc.vector.tensor_copy(k_sb[:, sl], pk)

                if ch == 0:
                    nc.sync.dma_start(wv_sb[:], wv[:])
                    nc.sync.dma_start(mb_sb[:], mb[:])
                    nc.sync.dma_start(one_sb[:], one[:])
                nc.sync.dma_start(xv_sb[:, ch], xv[ch])
                for kvs in range(4):
                    kv = ch * 4 + kvs
                    pv = ps_proj.tile([128, DV], F32, tag="proj")
                    for dt_i in range(NDT):
                        nc.tensor.matmul(
                            pv, xv_sb[:, ch, dt_i, kvs * 128:(kvs + 1) * 128],
                            wv_sb[:, dt_i, :],
                            start=(dt_i == 0), stop=(dt_i == NDT - 1),
                        )
                    nc.vector.tensor_copy(v_sb[:, kv, :], pv)

                if ch == 0:
                    nc.sync.dma_start(wq_sb[:], wq[:])
                nc.sync.dma_start(xq_sb[:, ch], xq[ch])
                for hi in range(NHEAD):
                    pq = ps_proj.tile([128, QC], F32, tag="proj")
                    for dt_i in range(NDT):
                        nc.tensor.matmul(
                            pq,
                            wq_sb[:, dt_i, hi * DQK:(hi + 1) * DQK],
                            xq_sb[:, ch, dt_i, :],
                            start=(dt_i == 0), stop=(dt_i == NDT - 1),
                        )
                    nc.vector.tensor_copy(q_sb[:, hi, sl], pq)

                # ---- attention for q chunk ch, two head-pair passes ----
                # between passes, emit next chunk's xk DMA early so its
                # k-projection can interleave into pass B's exp-bound stretch
                ctxT = ctpool.tile([128, NHEAD, QC], F16, tag="ctxT")
                nkv = 4 * ch + 4
                for pi in range(2):
                    if pi == 1 and ch + 1 < NQC:
                        nc.sync.dma_start(xk_sb[:, ch + 1], xk[ch + 1])
                    ctx2 = ps_ctx.tile([128, 2, QC], F32, tag="ctx")
                    acc = accpool.tile([128, 2, QC], F16, tag="acc")
                    for kv in range(nkv):
                        t = kv - 4 * ch
                        qoff = max(t, 0) * 128
                        s2 = ps_s.tile([128, 2, QC], F32, tag="s2")
                        for i in range(2):
                            h = pi * 2 + i
                            nc.tensor.matmul(
                                s2[:, i, qoff:QC],
                                k_sb[:, kv * 128:(kv + 1) * 128],
                                q_sb[:, h, ch * QC + qoff:(ch + 1) * QC],
                                start=True, stop=True,
                            )
                        # exp; first kv tile lands directly in acc
                        e2 = (acc if kv == 0
                              else epool.tile([128, 2, QC], F16, tag="e2"))
                        nc.scalar.activation(
                            e2[:, :, qoff:QC], s2[:, :, qoff:QC],
                            mybir.ActivationFunctionType.Exp,
                            bias=0.0, scale=scale_val,
                        )
                        if t >= 0:
                            # zero the below-diagonal of the 128-block
                            nc.vector.tensor_tensor(
                                e2[:, :, qoff:qoff + 128],
                                e2[:, :, qoff:qoff + 128], mb_sb[:],
                                mybir.AluOpType.mult,
                            )
                        last = kv == nkv - 1
                        for i in range(2):
                            nc.tensor.matmul(
                                ctx2[:, i, qoff:QC], v_sb[:, kv, :],
                                e2[:, i, qoff:QC],
                                start=(kv == 0), stop=last,
                            )
                        if kv > 0:
                            nc.vector.tensor_tensor(
                                acc[:, :, qoff:QC],
                                acc[:, :, qoff:QC], e2[:, :, qoff:QC],
                                mybir.AluOpType.add,
                            )
                    # pass-end: softmax denominators + normalize into ctxT
                    for i in range(2):
                        h = pi * 2 + i
                        alt = i == 1 and ch == NQC - 1
                        dpool = ps_proj if alt else ps_od
                        dps = dpool.tile(
                            [128, QC], F32, tag="proj" if alt else "od"
                        )
                        nc.tensor.matmul(
                            dps, one_sb[:], acc[:, i, :], start=True, stop=True
                        )
                        rb = rbpool.tile([128, QC], F32, tag="rb")
                        nc.vector.reciprocal_approx_fast(rb[:], dps[:])
                        nc.vector.tensor_tensor(
                            ctxT[:, h, :], ctx2[:, i, :], rb[:],
                            mybir.AluOpType.mult,
                        )

                # ---- out projection for this q chunk ----
                if ch == 0:
                    nc.sync.dma_start(wo_sb[:], wo[:])
                for j in range(4):
                    o_sb = opool.tile([128, D], F16, tag="o")
                    for nch in range(2):
                        alt = (j * 2 + nch) % 2 == 1 and ch == NQC - 1
                        opool_ps = ps_proj if alt else ps_od
                        po = opool_ps.tile(
                            [128, QC], F32, tag="proj" if alt else "od"
                        )
                        for hi in range(NHEAD):
                            nc.tensor.matmul(
                                po,
                                ctxT[:, hi, j * 128:(j + 1) * 128],
                                wo_sb[:, hi, nch * 512:(nch + 1) * 512],
                                start=(hi == 0), stop=(hi == NHEAD - 1),
                            )
                        nc.vector.tensor_copy(
                            o_sb[:, nch * 512:(nch + 1) * 512], po
                        )
                        qt = ch * 4 + j
                        nc.sync.dma_start(
                            out[qt * 128:(qt + 1) * 128,
                                nch * 512:(nch + 1) * 512],
                            o_sb[:, nch * 512:(nch + 1) * 512],
                        )

    nc.finalize()
    return nc


_NC_CACHE: dict[float, bass.Bass] = {}


def _get_nc(scale_val: float) -> bass.Bass:
    if scale_val not in _NC_CACHE:
        _NC_CACHE[scale_val] = _build(scale_val)
    return _NC_CACHE[scale_val]


def _chunk_tile(a: np.ndarray) -> np.ndarray:
    """[K, F] -> [F//QC, 128, K//128, QC] chunk-major partition-tiled fp16."""
    k, f = a.shape
    b = a.reshape(k // 128, 128, f // QC, QC)          # [po, pi, ch, qc]
    return np.ascontiguousarray(
        b.transpose(2, 1, 0, 3).astype(np.float16)     # [ch, pi, po, qc]
    )


def _part_tile(a: np.ndarray) -> np.ndarray:
    """[K, F] -> [128, K//128, F] partition-tiled fp16 contiguous."""
    k, f = a.shape
    return np.ascontiguousarray(
        a.reshape(k // 128, 128, f).transpose(1, 0, 2).astype(np.float16)
    )


def run(inputs: dict, trace: bool = False):
    in_q = np.asarray(inputs["in_q"], np.float32)
    in_k = np.asarray(inputs["in_k"], np.float32)
    in_v = np.asarray(inputs["in_v"], np.float32)
    Wq = np.asarray(inputs["Wq"], np.float32)
    Wk = np.asarray(inputs["Wk"], np.float32)
    Wv = np.asarray(inputs["Wv"], np.float32)
    Wo = np.asarray(inputs["Wo"], np.float32)
    bq = np.asarray(inputs["bq"], np.float32)
    bk = np.asarray(inputs["bk"], np.float32)
    bv = np.asarray(inputs["bv"], np.float32)
    bo = np.asarray(inputs["bo"], np.float32)
    qes = float(np.asarray(inputs["q_extra_scale"], np.float32).reshape(-1)[0])

    assert not (np.any(bq) or np.any(bk) or np.any(bv)), (
        "kernel compiled for zero qkv biases (reference constructs zeros)"
    )
    scale_val = qes / float(np.sqrt(DQK))
    nc = _get_nc(scale_val)

    # multiplicative causal mask for the diagonal 128x128 block
    ii = np.arange(128)[:, None]   # kv within tile (partition)
    jj = np.arange(128)[None, :]   # q within tile (free)
    mb1 = (jj >= ii).astype(np.float16)
    mb = np.ascontiguousarray(
        np.broadcast_to(mb1, (2, 128, 128)).transpose(1, 0, 2)
    )  # [128, 2, 128]
    ones = np.ones((128, 128), dtype=np.float16)

    in_maps = []
    for c in range(NCORES):
        b, g, hh = c // 4, (c % 4) // 2, c % 2
        h0 = g * HPG + hh * NHEAD
        wo_slice = Wo[h0 * DV:(h0 + NHEAD) * DV, :]  # [512, 1024]
        in_maps.append({
            "xqT": _chunk_tile(in_q[b].T),
            "xkT": _chunk_tile(in_k[b].T),
            "xvT": _chunk_tile(in_v[b].T),
            "wq": _part_tile(Wq[:, h0 * DQK:(h0 + NHEAD) * DQK]),
            "wk": _part_tile(Wk[:, g * DQK:(g + 1) * DQK]),
            "wv": _part_tile(Wv[:, g * DV:(g + 1) * DV]),
            "wo": np.ascontiguousarray(
                wo_slice.reshape(NHEAD, DV, D).transpose(1, 0, 2).astype(np.float16)
            ),
            "mb": mb,
            "one": ones,
        })

    res = run_bass_kernel_spmd(
        nc, in_maps, core_ids=list(range(NCORES)), trace=trace
    )

    out_full = np.zeros((B, L, D), np.float32)
    for c in range(NCORES):
        out_full[c // 4] += np.asarray(res.results[c]["out"], np.float32)
    out_full += bo
    return out_full, res.exec_time_ns


def kernel(**inputs) -> np.ndarray:
    out, _ = run(inputs, trace=False)
    return out

